# revision 1
# baseline (speedup 1.0000x reference)
"""ListMLE loss kernel for Trainium2 (8 NeuronCores, Bass/Tile).

loss = mean(logcumsumexp(outputs[t, labels[t]], axis=1) - outputs)

Strategy (per core, rows sharded 1024/core):
  - 8 row-tiles of [128, 4096]: exp on ACT, per-row gather via GPSIMD
    ap_gather (per-core index lists; subcall s of 16 covers rows p≡s mod 16,
    host pre-wraps label slices so each Q7 core gets the right row's labels),
    inclusive cumsum via DVE tensor_tensor_scan, Ln+accumulate on ACT.
  - Garbage partitions from the shared-index gather are masked out at the
    end with a host-supplied {0,1} mask before the final reduction.
  - Per-core partial = (sum ln scores - sum outputs) / (B*N); host sums the
    8 partials (the all-reduce of the sharding hint).
"""

import numpy as np

import concourse.bacc as bacc
import concourse.mybir as mybir
import concourse.tile as tile
from concourse.bass_utils import run_bass_kernel_spmd

B, N = 8192, 4096
N_CORES = 8
ROWS = B // N_CORES      # 1024 rows per core
TILES = ROWS // 128      # 8
SUB = 16                 # gather subcalls per tile (one per partition-mod-16)
NW = N // 16             # wrapped index columns

_NC = None


def _build():
    nc = bacc.Bacc("TRN2", target_bir_lowering=False, debug=False,
                   num_devices=N_CORES)
    O = nc.dram_tensor("outputs", [ROWS, N], mybir.dt.float32,
                       kind="ExternalInput").ap()
    LW = nc.dram_tensor("lblw", [TILES * SUB * 128, NW], mybir.dt.int16,
                        kind="ExternalInput").ap()
    MK = nc.dram_tensor("mask", [128, SUB], mybir.dt.float32,
                        kind="ExternalInput").ap()
    OUT = nc.dram_tensor("out", [1, 1], mybir.dt.float32,
                         kind="ExternalOutput").ap()
    f32 = mybir.dt.float32
    add = mybir.AluOpType.add

    with tile.TileContext(nc) as tc:
        with tc.tile_pool(name="main", bufs=2) as pool, \
             tc.tile_pool(name="small", bufs=1) as spool:
            acc_sc = spool.tile([128, SUB], f32, tag="acc_sc")
            acc_o = spool.tile([128, 1], f32, tag="acc_o")
            mask = spool.tile([128, SUB], f32, tag="mask")
            nc.vector.memset(acc_sc[:], 0.0)
            nc.vector.memset(acc_o[:], 0.0)
            nc.sync.dma_start(out=mask[:], in_=MK[:])
            for t in range(TILES):
                o = pool.tile([128, N], f32, tag="o")
                nc.sync.dma_start(out=o[:], in_=O[128 * t:128 * (t + 1), :])
                e = pool.tile([128, N], f32, tag="e")
                nc.scalar.activation(e[:], o[:],
                                     mybir.ActivationFunctionType.Exp)
                osum = pool.tile([128, 1], f32, tag="osum")
                nc.vector.tensor_reduce(osum[:], o[:],
                                        axis=mybir.AxisListType.X, op=add)
                nc.vector.tensor_tensor(out=acc_o[:], in0=acc_o[:],
                                        in1=osum[:], op=add)
                for s in range(SUB):
                    idx = pool.tile([128, NW], mybir.dt.int16, tag="idx")
                    base = (t * SUB + s) * 128
                    nc.sync.dma_start(out=idx[:], in_=LW[base:base + 128, :])
                    g = pool.tile([128, N], f32, tag="g")
                    nc.gpsimd.ap_gather(g[:], e[:], idx[:], channels=128,
                                        num_elems=N, d=1, num_idxs=N)
                    sc = pool.tile([128, N], f32, tag="sc")
                    nc.vector.tensor_tensor_scan(sc[:], g[:], g[:], 0.0,
                                                 add, mybir.AluOpType.bypass)
                    lnt = pool.tile([128, N], f32, tag="lnt")
                    lnacc = pool.tile([128, 1], f32, tag="lnacc")
                    nc.scalar.activation(lnt[:], sc[:],
                                         mybir.ActivationFunctionType.Ln,
                                         accum_out=lnacc[:])
                    nc.vector.tensor_tensor(out=acc_sc[:, s:s + 1],
                                            in0=acc_sc[:, s:s + 1],
                                            in1=lnacc[:], op=add)
            mrow = spool.tile([128, SUB], f32, tag="mrow")
            nc.vector.tensor_tensor(out=mrow[:], in0=acc_sc[:], in1=mask[:],
                                    op=mybir.AluOpType.mult)
            mred = spool.tile([128, 1], f32, tag="mred")
            nc.vector.tensor_reduce(mred[:], mrow[:],
                                    axis=mybir.AxisListType.X, op=add)
            comb = spool.tile([128, 1], f32, tag="comb")
            nc.vector.tensor_tensor(out=comb[:], in0=mred[:], in1=acc_o[:],
                                    op=mybir.AluOpType.subtract)
            tot = spool.tile([1, 1], f32, tag="tot")
            nc.gpsimd.tensor_reduce(tot[:], comb[:],
                                    axis=mybir.AxisListType.C, op=add)
            res = spool.tile([1, 1], f32, tag="res")
            nc.scalar.mul(res[:], tot[:], 1.0 / (B * N))
            nc.sync.dma_start(out=OUT[:], in_=res[:])
    nc.compile()
    return nc


def _get_nc():
    global _NC
    if _NC is None:
        _NC = _build()
    return _NC


def _prep_inputs(outputs, labels):
    outputs = np.ascontiguousarray(np.asarray(outputs), dtype=np.float32)
    lab16 = np.asarray(labels).astype(np.int16)  # values in [0, 4096)
    mask = (np.arange(128)[:, None] % 16 == np.arange(SUB)[None, :]) \
        .astype(np.float32)
    in_maps = []
    for c in range(N_CORES):
        Oc = outputs[c * ROWS:(c + 1) * ROWS]
        Lc = lab16[c * ROWS:(c + 1) * ROWS]
        # lw[(t*16+s)*128 + 16c + p', i'] = Lc[128t + 16c + s, 16i' + p']
        L5 = Lc.reshape(TILES, 8, 16, NW, 16)          # [t, cg, s, i', p']
        lw = np.ascontiguousarray(
            L5.transpose(0, 2, 1, 4, 3).reshape(TILES * SUB * 128, NW))
        in_maps.append({"outputs": Oc, "lblw": lw, "mask": mask})
    return in_maps


def kernel(outputs, labels):
    nc = _get_nc()
    in_maps = _prep_inputs(outputs, labels)
    res = run_bass_kernel_spmd(nc, in_maps, core_ids=list(range(N_CORES)))
    total = sum(float(r["out"][0, 0]) for r in res.results)
    return np.float32(total)



# revision 3
# speedup vs baseline: 1.0388x; 1.0388x over previous
"""ListMLE loss kernel for Trainium2 (8 NeuronCores, Bass/Tile).

loss = mean(logcumsumexp(outputs[t, labels[t]], axis=1) - outputs)

Per-row gather via per-partition local_scatter (GPSIMD streams at ~line
rate, unlike per-index ap_gather):
  host: counting-sort each row's labels in thirds (ints only) ->
        run-start offsets OFF_g and sort permutation PERM_g;
  device, per 128-row tile: E = exp(o) in bf16; per third g:
        T_g = local_scatter(E by OFF_g)      (E value at each run start)
        S_g = ttscan(mask*state + T_g)       (fill values through runs)
        G band = local_scatter(S_g by PERM_g) (back to original order;
                                              PERM is duplicate-free)
  then C = cumsum(G), ln(C) accumulated, minus sum(outputs); the 8
  per-core [128,1] partials are summed on host (the all-reduce).

The tile loop is software-pipelined: dma+exp for tile t+1 issue before
tile t's scatter chain so ACT's Ln(t) never blocks exp(t+1).
"""

import numpy as np

import concourse.bacc as bacc
import concourse.mybir as mybir
import concourse.tile as tile
import concourse.bass_isa as bass_isa
from concourse.bass_utils import run_bass_kernel_spmd

B, N = 8192, 4096
N_CORES = 8
ROWS = B // N_CORES          # 1024
TILES = ROWS // 128          # 8
THIRDS = [(0, 1366), (1366, 1366), (2732, 1364)]

_NC = None


def _local_scatter(gp, out_ap, data_ap, idxs_ap, num_elems, num_idxs):
    # like nc.gpsimd.local_scatter but allows any num_elems*32 <= 65472
    assert num_elems * 32 <= 65472 and num_elems % 2 == 0 and num_idxs % 2 == 0
    return gp.add_instruction(bass_isa.InstLocalScatter(
        name=f"I-{gp.bass.next_id()}",
        ins=[gp.lower_ap(data_ap, for_isa=True),
             gp.lower_ap(idxs_ap, for_isa=True)],
        outs=[gp.lower_ap(out_ap, for_isa=True)],
        _channels=128, _num_elems=num_elems, _num_idxs=num_idxs))


def _build():
    nc = bacc.Bacc("TRN2", target_bir_lowering=False, debug=False,
                   num_devices=N_CORES)
    f32 = mybir.dt.float32
    bf16 = mybir.dt.bfloat16
    i16 = mybir.dt.int16
    add = mybir.AluOpType.add

    O = nc.dram_tensor("outputs", [ROWS, N], f32, kind="ExternalInput").ap()
    OFFS = [nc.dram_tensor(f"off{g}", [ROWS, N], i16,
                           kind="ExternalInput").ap() for g in range(3)]
    PERMS = [nc.dram_tensor(f"perm{g}", [ROWS, sz], i16,
                            kind="ExternalInput").ap()
             for g, (st, sz) in enumerate(THIRDS)]
    OUT = nc.dram_tensor("out", [128, 1], f32, kind="ExternalOutput").ap()

    with tile.TileContext(nc) as tc:
        with tc.tile_pool(name="dma", bufs=2) as dpool, \
             tc.tile_pool(name="cmp", bufs=2) as cpool, \
             tc.tile_pool(name="sm", bufs=1) as spool:
            acc = spool.tile([128, 2], f32, name="acc")
            nc.vector.memset(acc[:], 0.0)

            state = {}

            def front(t):
                # dma + exp + osum for tile t (runs ahead of tile t-1's
                # scatter chain)
                r0 = 128 * t
                o = dpool.tile([128, N], f32, name="o", tag="o")
                nc.sync.dma_start(out=o[:], in_=O[r0:r0 + 128, :])
                offs = [dpool.tile([128, N], i16, name=f"offt{g}",
                                   tag=f"offt{g}") for g in range(3)]
                perms = [dpool.tile([128, sz], i16, name=f"permt{g}",
                                    tag=f"permt{g}")
                         for g, (st, sz) in enumerate(THIRDS)]
                for g, (st, sz) in enumerate(THIRDS):
                    nc.sync.dma_start(out=offs[g][:],
                                      in_=OFFS[g][r0:r0 + 128, :])
                    nc.sync.dma_start(out=perms[g][:],
                                      in_=PERMS[g][r0:r0 + 128, :])
                e = cpool.tile([128, N], bf16, name="e", tag="e")
                nc.scalar.activation(e[:], o[:],
                                     mybir.ActivationFunctionType.Exp)
                # sum(outputs) via ACT copy-accumulate (keeps DVE free);
                # the copy destination is scratch, reused as lnt later.
                trash = cpool.tile([128, N], bf16, name="trash", tag="lnt")
                osum = cpool.tile([128, 1], f32, name="osum", tag="osum")
                nc.scalar.activation(trash[:], o[:],
                                     mybir.ActivationFunctionType.Copy,
                                     accum_out=osum[:])
                nc.vector.tensor_tensor(out=acc[:, 1:2], in0=acc[:, 1:2],
                                        in1=osum[:], op=add)
                state[t] = (e, offs, perms)

            def back(t):
                # scatter chain + scan + ln for tile t
                e, offs, perms = state.pop(t)
                G = cpool.tile([128, N], bf16, name="G", tag="G")
                Ts, Ss = [], []
                for g, (st, sz) in enumerate(THIRDS):
                    T = cpool.tile([128, sz], bf16, name=f"T{g}",
                                   tag=f"T{g}")
                    _local_scatter(nc.gpsimd, T[:], e[:], offs[g][:],
                                   num_elems=sz, num_idxs=N)
                    Ts.append(T)
                for g, (st, sz) in enumerate(THIRDS):
                    a = cpool.tile([128, sz], bf16, name=f"a{g}", tag="a",
                                   padded_shape=[128, 1366])
                    nc.vector.tensor_scalar(out=a[:], in0=Ts[g][:],
                                            scalar1=0.0, scalar2=None,
                                            op0=mybir.AluOpType.is_equal)
                    S = cpool.tile([128, sz], bf16, name=f"S{g}",
                                   tag=f"S{g}")
                    nc.vector.tensor_tensor_scan(S[:], a[:], Ts[g][:], 0.0,
                                                 mybir.AluOpType.mult, add)
                    Ss.append(S)
                for g, (st, sz) in enumerate(THIRDS):
                    _local_scatter(nc.gpsimd, G[:, st:st + sz], Ss[g][:],
                                   perms[g][:], num_elems=sz, num_idxs=sz)
                C = cpool.tile([128, N], bf16, name="C", tag="C")
                nc.vector.tensor_tensor_scan(C[:], G[:], G[:], 0.0, add,
                                             mybir.AluOpType.bypass)
                lnt = cpool.tile([128, N], bf16, name="lnt2", tag="lnt")
                lnacc = cpool.tile([128, 1], f32, name="lnacc", tag="lnacc")
                nc.scalar.activation(lnt[:], C[:],
                                     mybir.ActivationFunctionType.Ln,
                                     accum_out=lnacc[:])
                nc.vector.tensor_tensor(out=acc[:, 0:1], in0=acc[:, 0:1],
                                        in1=lnacc[:], op=add)

            front(0)
            for t in range(TILES):
                if t + 1 < TILES:
                    front(t + 1)
                back(t)

            comb = spool.tile([128, 1], f32, name="comb")
            nc.vector.tensor_tensor(out=comb[:], in0=acc[:, 0:1],
                                    in1=acc[:, 1:2],
                                    op=mybir.AluOpType.subtract)
            nc.sync.dma_start(out=OUT[:], in_=comb[:])
    nc.compile()
    return nc


def _get_nc():
    global _NC
    if _NC is None:
        _NC = _build()
    return _NC


def _prep_inputs(outputs, labels):
    outputs = np.ascontiguousarray(np.asarray(outputs), dtype=np.float32)
    lab = np.asarray(labels).astype(np.int16)          # values in [0, 4096)
    # one radix argsort with key = label | third_id<<12 sorts each third
    key = lab.copy()
    key[:, THIRDS[1][0]:THIRDS[2][0]] += np.int16(1 << 12)
    key[:, THIRDS[2][0]:] += np.int16(2 << 12)
    si_full = np.argsort(key, axis=1, kind="stable")
    sk_full = np.sort(key, axis=1, kind="stable")

    offs, perms = [], []
    for g, (st, sz) in enumerate(THIRDS):
        si = (si_full[:, st:st + sz] - st).astype(np.int16)
        SL = (sk_full[:, st:st + sz] - np.int16(g << 12)).astype(np.int16)
        off = np.full((B, N), -1, dtype=np.int16)
        # write slots in descending order so the run START wins
        slots = np.broadcast_to(
            np.arange(sz - 1, -1, -1, dtype=np.int16), (B, sz))
        np.put_along_axis(off, SL[:, ::-1].astype(np.int64), slots, axis=1)
        offs.append(off)
        perms.append(si)

    in_maps = []
    for c in range(N_CORES):
        sl = slice(c * ROWS, (c + 1) * ROWS)
        m = {"outputs": outputs[sl]}
        for g in range(3):
            m[f"off{g}"] = offs[g][sl]
            m[f"perm{g}"] = perms[g][sl]
        in_maps.append(m)
    return in_maps


def kernel(outputs, labels):
    nc = _get_nc()
    in_maps = _prep_inputs(outputs, labels)
    res = run_bass_kernel_spmd(nc, in_maps, core_ids=list(range(N_CORES)))
    total = sum(float(r["out"].sum()) for r in res.results)
    return np.float32(total / (B * N))
